# revision 20
# baseline (speedup 1.0000x reference)
"""MGCN (3-layer RGCN-style message passing) on 8 Trainium2 NeuronCores.

Sharding: edges are sharded by destination-node range, aligned with a
node-range sharding of the output (core c owns nodes [c*NS, (c+1)*NS)).
Each core fully aggregates messages for its own nodes, so no all-reduce
is needed; an AllGather replicates the new node features between layers.

Gathers use the native SWDGE dma_gather (one instruction covers a
super-block of 7 dst blocks x one source-row group), with the node table
split into NG row groups of GRP<=32768 rows to satisfy the int16 index
format. Edges of each block are bucketed by source group into 128-edge
tiles (e-major); the per-(block,group) tile counts are maxed across
cores so the single SPMD program fits every core's data.

Per 128-node dst block:
  - one DVE is_equal builds all TM one-hot slot matrices
        O_n[e, t, m] = (iota[m] == slot_{e,t})
  - per edge tile t: one DVE broadcast-mult builds
        O_w[e, (b, m)] = O_n[e, t, m] * attE_{e,t,b}
    and one TensorE matmul accumulates Z^T[f,(b,m)] += Xg_t^T @ O_w
  - the block's own rows arrive by direct DMA from the core-local slab
    (they are contiguous); an identity matmul transposes them for root
The epilogue applies basis + root + bias with 6 accumulating matmuls
(bias rides a ones x bias/128 outer product); PSUM->SBUF copies and the
ReLU run on the otherwise idle Scalar engine.

Host side does index prep only: bucket/sort edges, build the wrapped
int16 gather indices, lay slot/att arrays out e-major (contiguous
loads), and cast dtypes. All feature FLOPs run on device.
"""

import math

import numpy as np
import ml_dtypes

import concourse.bass as bass
import concourse.tile as tile
from concourse import bacc, mybir
from concourse.bass_utils import run_bass_kernel_spmd
from concourse.library_config import mlp

P = 128
NBAS = 4
N_CORES = 8
GRP = 20096          # rows per gather source group (157*128, < 32768)
SBS = 7              # dst blocks per gather super-block
MAXT = 7             # max tiles (128 idxs each) per dma_gather (hw num_idxs cap)
POOL_TILES = 0       # edge tiles per block whose O_w builds run on GpSimd

BF16 = mybir.dt.bfloat16
F32 = mybir.dt.float32
I32 = mybir.dt.int32
I16 = mybir.dt.int16

_NP_OF = {BF16: ml_dtypes.bfloat16, F32: np.float32}


def _structure(nblk, NG, tiles):
    """Derive the static gather/tiling structure shared by all cores.

    tiles: [nblk][NG] tile counts (cross-core max).
    Returns (sbs, TM, per-block column lists, per-(sb,g) info, total idx len).
    """
    sbs = [(i, min(i + SBS, nblk)) for i in range(0, nblk, SBS)]
    T_b = [sum(tiles[b]) for b in range(nblk)]
    TM = max(T_b)
    blk_cols = [[] for _ in range(nblk)]   # per block: xgall column of tile j
    # per sb: (b0, b1, TOT, [(g, col0, n_tiles, idx_off)]) — one entry per
    # dma_gather instruction; (sb, g) runs longer than MAXT tiles are split
    sb_info = []
    idx_off = 0
    for b0, b1 in sbs:
        col = 0
        ginfo = []
        for g in range(NG):
            n_t = sum(tiles[b][g] for b in range(b0, b1))
            if n_t == 0:
                continue
            for b in range(b0, b1):
                for _ in range(tiles[b][g]):
                    blk_cols[b].append(col)
                    col += 1
            c0 = col - n_t
            while n_t > 0:
                n_c = min(n_t, MAXT)
                ginfo.append((g, c0, n_c, idx_off))
                idx_off += 128 * (n_c * 128 // 16)  # [128, S] int16 elements
                c0 += n_c
                n_t -= n_c
        sb_info.append((b0, b1, col, ginfo))
    return sbs, TM, T_b, blk_cols, sb_info, idx_off


def build_program(n_cores, nblk, NG, tiles, D=128, wdt=BF16):
    """Build the SPMD Bass program (same program for every core)."""
    NS = nblk * P
    NP_ = n_cores * NS
    nc = bacc.Bacc(num_devices=n_cores)
    Alu = mybir.AluOpType
    Act = mybir.ActivationFunctionType
    sbs, TM, T_b, blk_cols, sb_info, tot_idx = _structure(nblk, NG, tiles)

    x0 = nc.declare_dram_parameter("x0", [NP_, D], wdt, isOutput=False)
    x0loc = nc.declare_dram_parameter("x0loc", [NS, D], wdt, isOutput=False)
    idx_all = nc.declare_dram_parameter("idx_all", [tot_idx], I16, isOutput=False)
    # slotF = dst slot within block; attB1/2 = att*norm/deg (bf16)
    slotF = nc.declare_dram_parameter("slotF", [nblk, P, TM], F32, isOutput=False)
    attB1 = nc.declare_dram_parameter("attB1", [nblk, P, TM, NBAS], wdt, isOutput=False)
    attB2 = nc.declare_dram_parameter("attB2", [nblk, P, TM, NBAS], wdt, isOutput=False)
    basis5_1 = nc.declare_dram_parameter("basis5_1", [NBAS + 1, D, D], wdt, isOutput=False)
    basis5_2 = nc.declare_dram_parameter("basis5_2", [NBAS + 1, D, D], wdt, isOutput=False)
    biasd1 = nc.declare_dram_parameter("biasd1", [P, D], wdt, isOutput=False)
    biasd2 = nc.declare_dram_parameter("biasd2", [P, D], wdt, isOutput=False)
    iotaB = nc.declare_dram_parameter("iotaB", [P, P], F32, isOutput=False)
    identT = nc.declare_dram_parameter("identT", [P, P], wdt, isOutput=False)
    onesT = nc.declare_dram_parameter("onesT", [P, P], wdt, isOutput=False)
    outp = nc.declare_dram_parameter("out", [NS, D], F32, isOutput=True)

    # (attB, param-set index, relu)
    layers = [(attB1, 0, False), (attB1, 0, True), (attB2, 1, False)]

    with tile.TileContext(nc) as tc:
        with (
            tc.tile_pool(name="const", bufs=1) as cp,
            tc.tile_pool(name="sb", bufs=4) as sb,
            tc.tile_pool(name="idxp", bufs=2 * NG + 2) as idxp,
            tc.tile_pool(name="owp", bufs=6) as owp,
            tc.tile_pool(name="xgp", bufs=2) as xgp,
            tc.tile_pool(name="pp", bufs=2, space="PSUM") as pp,
            tc.tile_pool(name="dram", bufs=1, space="DRAM") as dp,
        ):
            nc.gpsimd.load_library(mlp)
            iota_sb = cp.tile([P, P], F32, tag="iota")
            nc.sync.dma_start(iota_sb[:], iotaB[:])
            ident_sb = cp.tile([P, P], wdt, tag="ident")
            nc.sync.dma_start(ident_sb[:], identT[:])
            ones_sb = cp.tile([P, P], wdt, tag="ones")
            nc.sync.dma_start(ones_sb[:], onesT[:])

            basis_sb = []
            biasd_sb = []
            for i, (b_h, bi_h) in enumerate(((basis5_1, biasd1), (basis5_2, biasd2))):
                bt = cp.tile([P, NBAS + 1, D], wdt, tag=f"basis{i}", name=f"basis_sb{i}")
                nc.sync.dma_start(bt[:], b_h[:].rearrange("b i o -> i b o"))
                basis_sb.append(bt)
                bit = cp.tile([P, D], wdt, tag=f"biasd{i}", name=f"biasd_sb{i}")
                nc.sync.dma_start(bit[:], bi_h[:])
                biasd_sb.append(bit)

            x_cur = x0
            xs_prev = x0loc
            for li, (attB, pi, relu) in enumerate(layers):
                last = li == len(layers) - 1
                if not last:
                    xs = dp.tile([NS, D], wdt, tag=f"xs{li}", name=f"xs{li}")
                    xnext = dp.tile(
                        [NP_, D], wdt, tag=f"xn{li}", name=f"xn{li}",
                        addr_space="Shared",
                    )
                for si, (b0, b1, TOT, ginfo) in enumerate(sb_info):
                    xgall = xgp.tile(
                        [P, TOT, D], wdt, tag="xg", name=f"xg_{li}_{si}"
                    )
                    for gi, (g, c0, n_t, ioff) in enumerate(ginfo):
                        n_idx = n_t * 128
                        S = n_idx // 16
                        idx_sb = idxp.tile(
                            [P, S], I16, tag="idx", name=f"idx_{li}_{si}_{gi}"
                        )
                        nc.sync.dma_start(
                            idx_sb[:],
                            idx_all[ioff : ioff + P * S].rearrange(
                                "(p s) -> p s", p=P
                            ),
                        )
                        glo = g * GRP
                        ghi = min((g + 1) * GRP, NP_)
                        nc.gpsimd.dma_gather(
                            xgall[:, c0 : c0 + n_t, :],
                            x_cur[glo:ghi, :],
                            idx_sb[:],
                            n_idx,
                            n_idx,
                            D,
                        )
                    for b in range(b0, b1):
                        slot_sb = sb.tile([P, TM], F32, tag="slot", name=f"sl_{li}_{b}")
                        nc.sync.dma_start(slot_sb[:], slotF[b])
                        att_sb = sb.tile(
                            [P, TM, NBAS], wdt, tag="att", name=f"at_{li}_{b}"
                        )
                        nc.sync.dma_start(att_sb[:], attB[b])
                        xself = sb.tile([P, D], wdt, tag="xself", name=f"xs_{li}_{b}")
                        nc.scalar.dma_start(
                            xself[:], xs_prev[b * P : (b + 1) * P, :]
                        )

                        # all one-hot slot matrices of the block in one DVE op
                        ona = owp.tile([P, TM, P], wdt, tag="ona", name=f"on_{li}_{b}")
                        nc.vector.tensor_tensor(
                            out=ona[:],
                            in0=iota_sb[:, None, :].to_broadcast([P, TM, P]),
                            in1=slot_sb[:, :, None].to_broadcast([P, TM, P]),
                            op=Alu.is_equal,
                        )

                        zps = pp.tile([P, NBAS, P], F32, tag="z", name=f"z_{li}_{b}")
                        nt = T_b[b]
                        for j in range(nt):
                            ow = owp.tile(
                                [P, NBAS, P], wdt, tag="ow", name=f"ow_{li}_{b}_{j}"
                            )
                            eng = nc.gpsimd if j < POOL_TILES else nc.vector
                            eng.tensor_tensor(
                                out=ow[:],
                                in0=ona[:, j, :][:, None, :].to_broadcast(
                                    [P, NBAS, P]
                                ),
                                in1=att_sb[:, j, :][:, :, None].to_broadcast(
                                    [P, NBAS, P]
                                ),
                                op=Alu.mult,
                            )
                            nc.tensor.matmul(
                                zps[:],
                                lhsT=xgall[:, blk_cols[b][j], :],
                                rhs=ow[:],
                                start=(j == 0),
                                stop=(j == nt - 1),
                            )
                        # self rows: x_blk^T via identity matmul (root term)
                        sps = pp.tile([P, P], F32, tag="s", name=f"s_{li}_{b}")
                        nc.tensor.matmul(
                            sps[:], lhsT=xself[:], rhs=ident_sb[:],
                            start=True, stop=True,
                        )

                        zt = sb.tile([P, NBAS, P], wdt, tag="zt", name=f"zt_{li}_{b}")
                        nc.scalar.copy(zt[:], zps[:])
                        xt = sb.tile([P, P], wdt, tag="xt", name=f"xt_{li}_{b}")
                        nc.scalar.copy(xt[:], sps[:])

                        agg = pp.tile([P, P], F32, tag="agg", name=f"agg_{li}_{b}")
                        for bb in range(NBAS):
                            nc.tensor.matmul(
                                agg[:],
                                lhsT=zt[:, bb, :],
                                rhs=basis_sb[pi][:, bb, :],
                                start=(bb == 0),
                                stop=False,
                            )
                        nc.tensor.matmul(
                            agg[:], lhsT=xt[:], rhs=basis_sb[pi][:, NBAS, :],
                            start=False, stop=False,
                        )
                        nc.tensor.matmul(
                            agg[:], lhsT=ones_sb[:], rhs=biasd_sb[pi][:],
                            start=False, stop=True,
                        )

                        ob = sb.tile(
                            [P, D],
                            F32 if last else wdt,
                            tag="ob_f" if last else "ob",
                            name=f"ob_{li}_{b}",
                        )
                        nc.scalar.activation(
                            ob[:], agg[:], Act.Relu if relu else Act.Copy
                        )
                        dst_rows = outp if last else xs
                        nc.sync.dma_start(dst_rows[b * P : (b + 1) * P, :], ob[:])
                if not last:
                    nc.gpsimd.collective_compute(
                        "AllGather",
                        Alu.bypass,
                        replica_groups=[list(range(n_cores))],
                        ins=[xs[:]],
                        outs=[xnext[:]],
                    )
                    x_cur = xnext
                    xs_prev = xs
    nc.compile()
    return nc


def prepare_inputs(
    entity, edge_index, edge_type, edge_norm, emb,
    att1, att2, basis1, basis2, root1, root2, bias1, bias2,
    n_cores=N_CORES, wdt=BF16,
):
    """Host-side index prep + sharding. Returns (in_maps, nblk, NG, tiles, N)."""
    npdt = _NP_OF[wdt]
    N = int(entity.shape[0])
    D = int(emb.shape[1])
    x_full = np.asarray(emb, np.float32)[np.asarray(entity, np.int64)]
    src = np.asarray(edge_index[0], np.int64)
    dst = np.asarray(edge_index[1], np.int64)
    et = np.asarray(edge_type, np.int64)
    norm = np.asarray(edge_norm, np.float32)

    NS = ((N + n_cores * P - 1) // (n_cores * P)) * P
    NP_ = NS * n_cores
    nblk = NS // P
    NG = (NP_ + GRP - 1) // GRP

    cnt = np.bincount(dst, minlength=NP_).astype(np.float32)
    nw_full = norm / np.maximum(cnt, 1.0)[dst]
    attE1_full = np.asarray(att1, np.float32)[et] * nw_full[:, None]
    attE2_full = np.asarray(att2, np.float32)[et] * nw_full[:, None]

    grp = src // GRP
    # sort edges by (dst block, src group); within a (block, group) run the
    # order is irrelevant
    gb_of = dst // P
    order = np.lexsort((grp, gb_of))
    nGB = NP_ // P

    # per (global block, group) counts -> cross-core max tile counts
    cell = gb_of * NG + grp
    ccnt = np.bincount(cell[order], minlength=nGB * NG).reshape(nGB, NG)
    ccnt_c = ccnt.reshape(n_cores, nblk, NG)
    tiles = tuple(
        tuple(
            int(max((-(-ccnt_c[c, b, g] // 128)) for c in range(n_cores)))
            for g in range(NG)
        )
        for b in range(nblk)
    )

    sbs, TM, T_b, blk_cols, sb_info, tot_idx = _structure(nblk, NG, tiles)

    # boundaries of each (block, group) run in the sorted edge list
    run_bounds = np.zeros(nGB * NG + 1, np.int64)
    np.cumsum(ccnt.reshape(-1), out=run_bounds[1:])

    x0 = np.zeros((NP_, D), np.float32)
    x0[:N] = x_full

    iotaB = np.tile(np.arange(P, dtype=np.float32), (P, 1))

    basis5_1 = np.concatenate(
        [np.asarray(basis1, np.float32), np.asarray(root1, np.float32)[None]], axis=0
    )
    basis5_2 = np.concatenate(
        [np.asarray(basis2, np.float32), np.asarray(root2, np.float32)[None]], axis=0
    )

    common = {
        "basis5_1": basis5_1.astype(npdt),
        "basis5_2": basis5_2.astype(npdt),
        "biasd1": np.tile(
            np.asarray(bias1, np.float32)[None, :] / P, (P, 1)
        ).astype(npdt),
        "biasd2": np.tile(
            np.asarray(bias2, np.float32)[None, :] / P, (P, 1)
        ).astype(npdt),
        "iotaB": iotaB,
        "identT": np.eye(P, dtype=np.float32).astype(npdt),
        "onesT": np.ones((P, P), np.float32).astype(npdt),
        "x0": x0.astype(npdt),
    }

    in_maps = []
    for c in range(n_cores):
        idx_arr = np.zeros(tot_idx, np.int16)
        slot_a = np.zeros((nblk, P, TM), np.float32)
        att1_a = np.zeros((nblk, P, TM, NBAS), np.float32)
        att2_a = np.zeros((nblk, P, TM, NBAS), np.float32)
        for b0, b1, TOT, ginfo in sb_info:
            # build each (sb, g) run's idx list and meta, then split the run
            # into the per-instruction chunks recorded in ginfo
            runs = {}
            for g in range(NG):
                n_t_all = sum(tiles[b][g] for b in range(b0, b1))
                if n_t_all == 0:
                    continue
                flat = np.zeros(n_t_all * 128, np.int16)
                pos = 0
                for b in range(b0, b1):
                    gb = c * nblk + b
                    lo, hi = run_bounds[gb * NG + g], run_bounds[gb * NG + g + 1]
                    sel = order[lo:hi]
                    k = hi - lo
                    flat[pos : pos + k] = src[sel] - g * GRP
                    # meta for these slots: tile j starts at this block's
                    # group-g tile base
                    jbase = sum(tiles[b][gg] for gg in range(g))
                    u = np.arange(k)
                    jj = jbase + u // 128
                    pp_ = u % 128
                    slot_a[b, pp_, jj] = dst[sel] - gb * P
                    att1_a[b, pp_, jj] = attE1_full[sel]
                    att2_a[b, pp_, jj] = attE2_full[sel]
                    pos += tiles[b][g] * 128
                runs[g] = (flat, 0)
            for g, c0, n_t, ioff in ginfo:
                flat, consumed = runs[g]
                chunk = flat[consumed * 128 : (consumed + n_t) * 128]
                runs[g] = (flat, consumed + n_t)
                S = n_t * 128 // 16
                wrapped = np.tile(chunk.reshape(S, 16).T, (8, 1))  # [128, S]
                idx_arr[ioff : ioff + P * S] = wrapped.reshape(-1)
        in_maps.append(
            dict(
                common,
                x0loc=np.ascontiguousarray(
                    x0[c * NS : (c + 1) * NS].astype(npdt)
                ),
                idx_all=idx_arr,
                slotF=slot_a,
                attB1=att1_a.astype(npdt),
                attB2=att2_a.astype(npdt),
            )
        )
    return in_maps, nblk, NG, tiles, N


_PROGRAM_CACHE = {}


def run(inputs_dict, n_cores=N_CORES, wdt=BF16, trace=False, trace_kwargs=None):
    """Full pipeline: prep, (cached) build, run, unshard. Returns (out, results)."""
    in_maps, nblk, NG, tiles, N = prepare_inputs(
        inputs_dict["entity"], inputs_dict["edge_index"], inputs_dict["edge_type"],
        inputs_dict["edge_norm"], inputs_dict["emb"],
        inputs_dict["att1"], inputs_dict["att2"],
        inputs_dict["basis1"], inputs_dict["basis2"],
        inputs_dict["root1"], inputs_dict["root2"],
        inputs_dict["bias1"], inputs_dict["bias2"],
        n_cores=n_cores, wdt=wdt,
    )
    key = (n_cores, nblk, NG, tiles, wdt)
    if key not in _PROGRAM_CACHE:
        _PROGRAM_CACHE[key] = build_program(n_cores, nblk, NG, tiles, wdt=wdt)
    nc = _PROGRAM_CACHE[key]
    kwargs = {}
    if trace:
        kwargs["trace"] = True
        if trace_kwargs:
            kwargs.update(trace_kwargs)
    res = run_bass_kernel_spmd(nc, in_maps, list(range(n_cores)), **kwargs)
    out = np.concatenate([res.results[c]["out"] for c in range(n_cores)], axis=0)[:N]
    return np.ascontiguousarray(out, dtype=np.float32), res


def kernel(**inputs):
    out, _ = run(inputs)
    return out


# revision 27
# speedup vs baseline: 1.2445x; 1.2445x over previous
"""MGCN (3-layer RGCN-style message passing) on 8 Trainium2 NeuronCores.

Sharding: edges are sharded by destination-node range, aligned with a
node-range sharding of the output (core c owns nodes [c*NS, (c+1)*NS)).
Each core fully aggregates messages for its own nodes, so no all-reduce
is needed; an AllGather replicates the new node features between layers.

Per 128-node block, per 128-edge tile (edges sorted by dst):
  - indirect-DMA gather of source features Xg [128e, 128f]
  - DVE builds O_n[e,m] = (m == slot_e) * nw_e        (one dual-op instr)
        and O_w[e,(b,m)] = O_n[e,m] * att_e[b]        (one bcast instr)
  - one TensorE matmul accumulates Z^T[f,(b,m)] += Xg^T @ O_w in PSUM
A "self tile" gathers the block's own rows and multiplies by identity,
yielding x_blk^T for the root term. The epilogue applies the basis and
root matrices with 5 accumulating matmuls, adds bias (+ReLU on layer 2),
and writes the block's output rows.

Host side does index prep only: sort edges by dst, tile/pad, gather the
tiny att[edge_type] table rows, fold 1/deg into the edge norm, and cast
dtypes. All feature FLOPs run on device.
"""

import math

import numpy as np
import ml_dtypes

import concourse.bass as bass
import concourse.tile as tile
from concourse import bacc, mybir
from concourse.bass_utils import run_bass_kernel_spmd

P = 128
NBAS = 4
N_CORES = 8

BF16 = mybir.dt.bfloat16
F32 = mybir.dt.float32
I32 = mybir.dt.int32

_NP_OF = {BF16: ml_dtypes.bfloat16, F32: np.float32}


def build_program(n_cores, nblk, T, D=128, wdt=BF16):
    """Build the SPMD Bass program (same program for every core)."""
    NS = nblk * P
    NP_ = n_cores * NS
    nc = bacc.Bacc(num_devices=n_cores)
    Alu = mybir.AluOpType

    x0 = nc.declare_dram_parameter("x0", [NP_, D], wdt, isOutput=False)
    x0loc = nc.declare_dram_parameter("x0loc", [NS, D], wdt, isOutput=False)
    # offs column T holds the block's own node ids (self/root gather)
    offs = nc.declare_dram_parameter("offs", [nblk, T + 1, P], I32, isOutput=False)
    slot = nc.declare_dram_parameter("slot", [nblk, T, P], F32, isOutput=False)
    # attE* carry att[edge_type] * edge_norm / deg(dst), pre-folded on host
    attE1 = nc.declare_dram_parameter("attE1", [nblk, T, P, NBAS], wdt, isOutput=False)
    attE2 = nc.declare_dram_parameter("attE2", [nblk, T, P, NBAS], wdt, isOutput=False)
    basis1 = nc.declare_dram_parameter("basis1", [NBAS, D, D], wdt, isOutput=False)
    basis2 = nc.declare_dram_parameter("basis2", [NBAS, D, D], wdt, isOutput=False)
    root1 = nc.declare_dram_parameter("root1", [D, D], wdt, isOutput=False)
    root2 = nc.declare_dram_parameter("root2", [D, D], wdt, isOutput=False)
    biasT1 = nc.declare_dram_parameter("biasT1", [P, D], wdt, isOutput=False)
    biasT2 = nc.declare_dram_parameter("biasT2", [P, D], wdt, isOutput=False)
    iotaT = nc.declare_dram_parameter("iotaT", [P, P], F32, isOutput=False)
    identT = nc.declare_dram_parameter("identT", [P, P], wdt, isOutput=False)
    outp = nc.declare_dram_parameter("out", [NS, D], F32, isOutput=True)

    # (attE, param-set index, relu)
    layers = [(attE1, 0, False), (attE1, 0, True), (attE2, 1, False)]

    with tile.TileContext(nc) as tc:
        with (
            tc.tile_pool(name="const", bufs=1) as cp,
            tc.tile_pool(name="sb", bufs=4) as sb,
            tc.tile_pool(name="xgp", bufs=6) as xgp,
            tc.tile_pool(name="pp", bufs=2, space="PSUM") as pp,
            tc.tile_pool(name="dram", bufs=1, space="DRAM") as dp,
        ):
            iota_sb = cp.tile([P, P], F32, tag="iota")
            nc.sync.dma_start(iota_sb[:], iotaT[:])
            ident_sb = cp.tile([P, P], wdt, tag="ident")
            nc.sync.dma_start(ident_sb[:], identT[:])

            basis_sb = []
            root_sb = []
            bias_sb = []
            for i, (b_h, r_h, bi_h) in enumerate(
                ((basis1, root1, biasT1), (basis2, root2, biasT2))
            ):
                bt = cp.tile([P, NBAS, D], wdt, tag=f"basis{i}", name=f"basis_sb{i}")
                nc.sync.dma_start(bt[:], b_h[:].rearrange("b i o -> i b o"))
                basis_sb.append(bt)
                rt = cp.tile([P, D], wdt, tag=f"root{i}", name=f"root_sb{i}")
                nc.sync.dma_start(rt[:], r_h[:])
                root_sb.append(rt)
                bit = cp.tile([P, D], wdt, tag=f"bias{i}", name=f"bias_sb{i}")
                nc.sync.dma_start(bit[:], bi_h[:])
                bias_sb.append(bit)

            x_cur = x0
            xs_prev = x0loc
            for li, (attE, pi, relu) in enumerate(layers):
                last = li == len(layers) - 1
                if not last:
                    xs = dp.tile([NS, D], wdt, tag=f"xs{li}", name=f"xs{li}")
                    xnext = dp.tile(
                        [NP_, D], wdt, tag=f"xn{li}", name=f"xn{li}",
                        addr_space="Shared",
                    )
                for nb in range(nblk):
                    offs_sb = sb.tile(
                        [P, T + 1], I32, tag="offs", name=f"offs_{li}_{nb}"
                    )
                    nc.sync.dma_start(offs_sb[:], offs[nb].rearrange("t e -> e t"))
                    slot_sb = sb.tile([P, T], F32, tag="slot", name=f"slot_{li}_{nb}")
                    nc.sync.dma_start(slot_sb[:], slot[nb].rearrange("t e -> e t"))
                    attE_sb = sb.tile(
                        [P, T, NBAS], wdt, tag="attE", name=f"attE_{li}_{nb}"
                    )
                    nc.sync.dma_start(attE_sb[:], attE[nb].rearrange("t e b -> e t b"))

                    # all T one-hot slot matrices of the block in one DVE op
                    ona = sb.tile([P, T, P], wdt, tag="ona", name=f"ona_{li}_{nb}")
                    nc.vector.tensor_tensor(
                        out=ona[:],
                        in0=iota_sb[:, None, :].to_broadcast([P, T, P]),
                        in1=slot_sb[:, :, None].to_broadcast([P, T, P]),
                        op=Alu.is_equal,
                    )

                    zps = pp.tile([P, NBAS, P], F32, tag="z", name=f"z_{li}_{nb}")
                    for t in range(T):
                        xgt = xgp.tile([P, D], wdt, tag="xg", name=f"xg_{li}_{nb}_{t}")
                        nc.gpsimd.indirect_dma_start(
                            out=xgt[:],
                            out_offset=None,
                            in_=x_cur[:, :],
                            in_offset=bass.IndirectOffsetOnAxis(
                                ap=offs_sb[:, t : t + 1], axis=0
                            ),
                        )
                        xg = xgt[:]
                        ow = sb.tile(
                            [P, NBAS, P], wdt, tag="ow", name=f"ow_{li}_{nb}_{t}"
                        )
                        nc.vector.tensor_tensor(
                            out=ow[:],
                            in0=ona[:, t, :][:, None, :].to_broadcast([P, NBAS, P]),
                            in1=attE_sb[:, t, :][:, :, None].to_broadcast(
                                [P, NBAS, P]
                            ),
                            op=Alu.mult,
                        )
                        nc.tensor.matmul(
                            zps[:],
                            lhsT=xg,
                            rhs=ow[:],
                            start=(t == 0),
                            stop=(t == T - 1),
                        )
                    # self tile: the block's own rows are contiguous in the
                    # core-local slab — direct DMA, no indirect gather
                    xgs = xgp.tile([P, D], wdt, tag="xgs", name=f"xgs_{li}_{nb}")
                    nc.scalar.dma_start(xgs[:], xs_prev[nb * P : (nb + 1) * P, :])
                    sps = pp.tile([P, P], F32, tag="s", name=f"s_{li}_{nb}")
                    nc.tensor.matmul(
                        sps[:], lhsT=xgs[:], rhs=ident_sb[:], start=True, stop=True
                    )

                    zt = sb.tile([P, NBAS, P], wdt, tag="zt", name=f"zt_{li}_{nb}")
                    nc.scalar.copy(zt[:], zps[:])
                    xt = sb.tile([P, P], wdt, tag="xt", name=f"xt_{li}_{nb}")
                    nc.scalar.copy(xt[:], sps[:])

                    agg = pp.tile([P, P], F32, tag="agg", name=f"agg_{li}_{nb}")
                    for b in range(NBAS):
                        nc.tensor.matmul(
                            agg[:],
                            lhsT=zt[:, b, :],
                            rhs=basis_sb[pi][:, b, :],
                            start=(b == 0),
                            stop=False,
                        )
                    nc.tensor.matmul(
                        agg[:], lhsT=xt[:], rhs=root_sb[pi][:], start=False, stop=True
                    )

                    ob = sb.tile(
                        [P, D],
                        F32 if last else wdt,
                        tag="ob_f" if last else "ob",
                        name=f"ob_{li}_{nb}",
                    )
                    nc.vector.tensor_tensor(
                        out=ob[:], in0=agg[:], in1=bias_sb[pi][:], op=Alu.add
                    )
                    if relu:
                        nc.vector.tensor_scalar(
                            out=ob[:],
                            in0=ob[:],
                            scalar1=0.0,
                            scalar2=None,
                            op0=Alu.max,
                        )
                    dst_rows = outp if last else xs
                    nc.sync.dma_start(dst_rows[nb * P : (nb + 1) * P, :], ob[:])
                if not last:
                    nc.gpsimd.collective_compute(
                        "AllGather",
                        Alu.bypass,
                        replica_groups=[list(range(n_cores))],
                        ins=[xs[:]],
                        outs=[xnext[:]],
                    )
                    x_cur = xnext
                    xs_prev = xs
    nc.compile()
    return nc


def prepare_inputs(
    entity, edge_index, edge_type, edge_norm, emb,
    att1, att2, basis1, basis2, root1, root2, bias1, bias2,
    n_cores=N_CORES, wdt=BF16,
):
    """Host-side index prep + sharding. Returns (in_maps, nblk, T, N, NS)."""
    npdt = _NP_OF[wdt]
    N = int(entity.shape[0])
    D = int(emb.shape[1])
    x_full = np.asarray(emb, np.float32)[np.asarray(entity, np.int64)]
    src = np.asarray(edge_index[0], np.int64)
    dst = np.asarray(edge_index[1], np.int64)
    et = np.asarray(edge_type, np.int64)
    norm = np.asarray(edge_norm, np.float32)

    NS = ((N + n_cores * P - 1) // (n_cores * P)) * P
    NP_ = NS * n_cores
    nblk = NS // P

    cnt = np.bincount(dst, minlength=NP_).astype(np.float32)
    nw_full = norm / np.maximum(cnt, 1.0)[dst]
    attE1_full = np.asarray(att1, np.float32)[et] * nw_full[:, None]
    attE2_full = np.asarray(att2, np.float32)[et] * nw_full[:, None]

    order = np.argsort(dst, kind="stable")
    gb_bounds = np.searchsorted(dst[order], np.arange(0, NP_ + 1, P))
    ecnt = np.diff(gb_bounds)
    T = max(1, int(math.ceil(ecnt.max() / P)))

    nGB = NP_ // P
    offs_a = np.zeros((nGB, T * P), np.int32)
    slot_a = np.zeros((nGB, T * P), np.float32)
    at1_a = np.zeros((nGB, T * P, NBAS), np.float32)
    at2_a = np.zeros((nGB, T * P, NBAS), np.float32)
    for gb in range(nGB):
        lo, hi = gb_bounds[gb], gb_bounds[gb + 1]
        k = hi - lo
        if k == 0:
            continue
        sel = order[lo:hi]
        offs_a[gb, :k] = src[sel]
        slot_a[gb, :k] = dst[sel] - gb * P
        at1_a[gb, :k] = attE1_full[sel]
        at2_a[gb, :k] = attE2_full[sel]

    x0 = np.zeros((NP_, D), np.float32)
    x0[:N] = x_full

    iotaT = np.tile(np.arange(P, dtype=np.float32), (P, 1))
    identT = np.eye(P, dtype=np.float32)

    common = {
        "basis1": np.asarray(basis1, np.float32).astype(npdt),
        "basis2": np.asarray(basis2, np.float32).astype(npdt),
        "root1": np.asarray(root1, np.float32).astype(npdt),
        "root2": np.asarray(root2, np.float32).astype(npdt),
        "biasT1": np.tile(np.asarray(bias1, np.float32)[None, :], (P, 1)).astype(npdt),
        "biasT2": np.tile(np.asarray(bias2, np.float32)[None, :], (P, 1)).astype(npdt),
        "iotaT": iotaT,
        "identT": identT.astype(npdt),
        "x0": x0.astype(npdt),
    }

    in_maps = []
    for c in range(n_cores):
        s = slice(c * nblk, (c + 1) * nblk)
        offs_c = np.concatenate(
            [
                offs_a[s].reshape(nblk, T, P),
                (c * NS + np.arange(NS, dtype=np.int32)).reshape(nblk, 1, P),
            ],
            axis=1,
        )
        in_maps.append(
            dict(
                common,
                x0loc=np.ascontiguousarray(x0[c * NS : (c + 1) * NS].astype(npdt)),
                offs=np.ascontiguousarray(offs_c),
                slot=slot_a[s].reshape(nblk, T, P),
                attE1=at1_a[s].reshape(nblk, T, P, NBAS).astype(npdt),
                attE2=at2_a[s].reshape(nblk, T, P, NBAS).astype(npdt),
            )
        )
    return in_maps, nblk, T, N, NS


_PROGRAM_CACHE = {}


def run(inputs_dict, n_cores=N_CORES, wdt=BF16, trace=False, trace_kwargs=None):
    """Full pipeline: prep, (cached) build, run, unshard. Returns (out, results)."""
    in_maps, nblk, T, N, NS = prepare_inputs(
        inputs_dict["entity"], inputs_dict["edge_index"], inputs_dict["edge_type"],
        inputs_dict["edge_norm"], inputs_dict["emb"],
        inputs_dict["att1"], inputs_dict["att2"],
        inputs_dict["basis1"], inputs_dict["basis2"],
        inputs_dict["root1"], inputs_dict["root2"],
        inputs_dict["bias1"], inputs_dict["bias2"],
        n_cores=n_cores, wdt=wdt,
    )
    key = (n_cores, nblk, T, wdt)
    if key not in _PROGRAM_CACHE:
        _PROGRAM_CACHE[key] = build_program(n_cores, nblk, T, wdt=wdt)
    nc = _PROGRAM_CACHE[key]
    kwargs = {}
    if trace:
        kwargs["trace"] = True
        if trace_kwargs:
            kwargs.update(trace_kwargs)
    res = run_bass_kernel_spmd(nc, in_maps, list(range(n_cores)), **kwargs)
    out = np.concatenate([res.results[c]["out"] for c in range(n_cores)], axis=0)[:N]
    return np.ascontiguousarray(out, dtype=np.float32), res


def kernel(**inputs):
    out, _ = run(inputs)
    return out
